# revision 20
# baseline (speedup 1.0000x reference)
"""Llama attention (B=2, S=2048, E=4096, H=32) on 8 trn2 NeuronCores.

Strategy (tensor-parallel over heads, 4 heads/core, all-bf16 datapath):
  - RoPE is position-independent here (cos/sin are [H, D/2]), so it is folded
    into wq/wk on the host; the 1/sqrt(D) scale is folded into wq too.
  - All matmul operands are bf16 (1 cycle/row on the PE, fast weight load),
    accumulation in fp32 PSUM.  The full E=4096 contraction of the Q/K/V
    projections accumulates in PSUM (32 chained matmuls per output tile), so
    no vector-engine adds are needed; a single scalar-engine copy (with bf16
    cast) evacuates each tile.
  - Scores are computed transposed (S^T = K^T-tile @ Q^T) and the attention
    output as O^T = V-tile @ P^T: zero on-device transposes.  exp() runs on
    the scalar engine over [128,1024] PSUM spans.  Softmax denominators:
    one wide bf16 DVE add per key-group -> gpsimd partition_all_reduce ->
    reciprocal_approx_fast -> one DVE multiply.  Softmax needs no
    max-subtraction (scores bounded ~ +-8; exp cannot overflow in fp32).
  - Per-core output is a partial Y (row-sharded wo) written bf16; the host
    sums the 8 partials in fp32.
"""

import sys

sys.path.insert(0, "/opt/trn_rl_repo")

import numpy as np
import ml_dtypes

B, S, E, H = 2, 2048, 4096, 32
D = 128            # head dim
NCORES = 8
HL = H // NCORES   # heads per core = 4
W = HL * D         # per-core projection width = 512
T = B * S          # 4096 tokens
NKB = 32           # 128-row contraction tiles over E
CH1 = 256          # phase-1 token chunk
NCH1 = S // CH1    # 8 chunks per batch
NTT = S // 128     # 16 token/key tiles per batch

_CACHE = {}


def _build_nc():
    import concourse.bass as bass  # noqa: F401
    import concourse.mybir as mybir
    import concourse.tile as tile
    from concourse import bacc
    from concourse.bass_isa import ReduceOp

    fp32 = mybir.dt.float32
    bf16 = mybir.dt.bfloat16
    EXP = mybir.ActivationFunctionType.Exp

    nc = bacc.Bacc("TRN2", target_bir_lowering=False, debug=False)

    xh_d = nc.dram_tensor("xh", [T // CH1, 128, NKB, CH1], bf16, kind="ExternalInput")
    wqk_d = nc.dram_tensor("wqk", [128, NKB, 2 * W], bf16, kind="ExternalInput")
    wv_d = nc.dram_tensor("wv", [128, NKB, W], bf16, kind="ExternalInput")
    wo_d = nc.dram_tensor("wo", [128, HL, E], bf16, kind="ExternalInput")
    y_d = nc.dram_tensor("y", [T, E], bf16, kind="ExternalOutput")

    with nc.allow_low_precision(reason="bf16 datapath; fp32 PSUM accumulation"), \
         tile.TileContext(nc) as tc:
        with tc.tile_pool(name="const", bufs=1) as constp, \
             tc.tile_pool(name="gw", bufs=1) as gwp, \
             tc.tile_pool(name="gwo", bufs=1) as wop:
            zbias = constp.tile([128, 1], fp32, tag="zbias")
            nc.vector.memset(zbias[:], 0.0)
            # pre-warm the ACT exp table during startup so the first real
            # exp in phase 2 doesn't pay the table load
            warm = constp.tile([128, 1], fp32, tag="warm")
            nc.scalar.activation(warm[:], zbias[:], EXP, bias=zbias[:, 0:1])

            wqk_t = gwp.tile([128, NKB, 2 * W], bf16, tag="wqk")
            wv_t = gwp.tile([128, NKB, W], bf16, tag="wv")

            for b in range(B):
                with tc.tile_pool(name=f"bt{b}", bufs=1) as btp:
                    QKT = btp.tile([128, 2 * HL, S], bf16, tag="qkt", name="qkt")
                    V = btp.tile([128, NTT, W], bf16, tag="v", name="v")
                    # m-major: OTT[:, m, kd, :] is the phase-3 stationary
                    OTT = btp.tile([128, NTT, HL, 128], bf16, tag="ott",
                                   name="ott")

                    # ---------------- phase 1: projections ----------------
                    with nc.named_scope(f"ph1b{b}"), \
                         tc.tile_pool(name=f"p1x{b}", bufs=2) as xpool, \
                         tc.tile_pool(name=f"p1qk{b}", bufs=1, space="PSUM") as psqk, \
                         tc.tile_pool(name=f"p1v{b}", bufs=2, space="PSUM") as psv:
                        for c in range(NCH1):
                            xn = xpool.tile([128, NKB, CH1], bf16, tag="xn")
                            if b == 0 and c == 0:
                                # fine-grained first loads, wv interleaved, so
                                # the PE starts early and V never stalls
                                nc.sync.dma_start(xn[:, 0:4, :],
                                                  xh_d[0, :, 0:4, :])
                                nc.sync.dma_start(wqk_t[:, 0:4, :],
                                                  wqk_d[:, 0:4, :])
                                nc.sync.dma_start(xn[:, 4:8, :],
                                                  xh_d[0, :, 4:8, :])
                                nc.sync.dma_start(wqk_t[:, 4:8, :],
                                                  wqk_d[:, 4:8, :])
                                nc.sync.dma_start(xn[:, 8:NKB, :],
                                                  xh_d[0, :, 8:NKB, :])
                                nc.sync.dma_start(wqk_t[:, 8:16, :],
                                                  wqk_d[:, 8:16, :])
                                nc.sync.dma_start(wv_t[:, 0:8, :],
                                                  wv_d[:, 0:8, :])
                                nc.sync.dma_start(wqk_t[:, 16:24, :],
                                                  wqk_d[:, 16:24, :])
                                nc.sync.dma_start(wv_t[:, 8:16, :],
                                                  wv_d[:, 8:16, :])
                                nc.sync.dma_start(wqk_t[:, 24:32, :],
                                                  wqk_d[:, 24:32, :])
                                nc.sync.dma_start(wv_t[:, 16:32, :],
                                                  wv_d[:, 16:32, :])
                            else:
                                nc.sync.dma_start(xn[:], xh_d[b * NCH1 + c])
                            pqk = psqk.tile([128, 2 * HL, CH1], fp32, tag="pqk",
                                            name="pqk")
                            pv = psv.tile([128, 2, W], fp32, tag="pv", name="pv")

                            def emit_v():
                                for kb in range(NKB):
                                    for ts in range(2):
                                        nc.tensor.matmul(
                                            pv[:, ts, :],
                                            xn[:, kb, ts * 128:(ts + 1) * 128],
                                            wv_t[:, kb, :],
                                            start=(kb == 0), stop=(kb == NKB - 1),
                                        )

                            def emit_qk():
                                for kb in range(NKB):
                                    for t in range(2 * HL):
                                        # two [128,256] tiles share one PSUM
                                        # bank; start clears the WHOLE bank's
                                        # has_written bits, so only the first
                                        # matmul touching each bank may carry
                                        # start=True.
                                        nc.tensor.matmul(
                                            pqk[:, t, :],
                                            wqk_t[:, kb, t * 128:(t + 1) * 128],
                                            xn[:, kb, :],
                                            start=(kb == 0 and t % 2 == 0),
                                            stop=(kb == NKB - 1 and t % 2 == 1),
                                        )

                            if c == 0:
                                emit_qk()
                                emit_v()
                            else:
                                emit_v()
                                emit_qk()
                            nc.scalar.copy(
                                QKT[:, :, c * CH1:(c + 1) * CH1], pqk[:])
                            nc.scalar.copy(V[:, 2 * c:2 * c + 2, :], pv[:])

                    # ---------------- phase 2: attention ----------------
                    # chunk = (head h, 512-query block sq); the denominator
                    # tail is split: merge+all_reduce right after its chunk,
                    # recip+multiply one chunk later, so neither the PE nor
                    # the in-order DVE queue ever waits on the gpsimd reduce.
                    wo_sl = [None] * 8
                    with nc.named_scope(f"ph2b{b}"), \
                         tc.tile_pool(name=f"a2e{b}", bufs=6) as ep, \
                         tc.tile_pool(name=f"a2s{b}", bufs=2) as esp, \
                         tc.tile_pool(name=f"a2r{b}", bufs=2) as rcp, \
                         tc.tile_pool(name=f"a2ps{b}", bufs=3, space="PSUM") as psS, \
                         tc.tile_pool(name=f"a2po{b}", bufs=2, space="PSUM") as psO:
                        for i in range(2):
                            wo_sl[i] = wop.tile([128, HL, 512], bf16,
                                                tag=f"wo{i}", name=f"wo{b}_{i}")
                            nc.sync.dma_start(
                                wo_sl[i][:], wo_d[:, :, i * 512:(i + 1) * 512])

                        state = {}
                        pend_tail = []   # (due_k, ci)

                        def emit_tail_head(ci):
                            po, esAB, _ = state[ci]
                            esum = rcp.tile([128, 512], fp32, tag="esum")
                            nc.vector.tensor_add(esum[:], esAB[:, 0, :],
                                                 esAB[:, 1, :])
                            denomB = rcp.tile([128, 4, 128], fp32, tag="denomB")
                            nc.gpsimd.partition_all_reduce(
                                denomB[:], esum[:], 128, ReduceOp.add)
                            state[ci][2] = denomB

                        def emit_tail_end(ci):
                            h, sq = ci
                            po, esAB, denomB = state.pop(ci)
                            rr = rcp.tile([128, 4, 128], fp32, tag="rr")
                            nc.vector.reciprocal_approx_fast(rr[:], denomB[:])
                            nc.vector.tensor_mul(
                                OTT[:, 4 * sq:4 * sq + 4, h, :], po[:], rr[:])

                        def consume(k, ci, g, eS):
                            h, sq = ci
                            po, esAB, _ = state[ci]
                            for j in range(2):
                                sk = 2 * g + j
                                nc.tensor.matmul(
                                    po[:],
                                    V[:, sk, h * 128:(h + 1) * 128],
                                    eS[:, j, :],
                                    start=(sk == 0), stop=(sk == 15),
                                )
                            if g == 0:
                                nc.vector.tensor_copy(esAB[:], eS[:])
                            else:
                                nc.vector.tensor_add(esAB[:], esAB[:], eS[:])
                            if g == 7:
                                emit_tail_head(ci)
                                pend_tail.append((k + 2, ci))

                        # flat group stream, PV consumption lagging LAG groups
                        # behind the pS/exp production so the PE never waits
                        # on a fresh exp (the in-order queue always has pS
                        # work between an exp and its PV consumer)
                        LAG = 2
                        chunks = [(h, sq) for sq in range(4) for h in range(HL)]
                        stream = [(ci, g) for ci in chunks for g in range(8)]
                        fifo = []
                        for k, (ci, g) in enumerate(stream):
                            h, sq = ci
                            q0 = sq * 512
                            if g == 0:
                                po = psO.tile([128, 4, 128], fp32, tag="po",
                                              name="po")
                                esAB = esp.tile([128, 2, 512], bf16,
                                                tag="esAB")
                                state[ci] = [po, esAB, None]
                            pS = psS.tile([128, 2, 512], fp32, tag="pS",
                                          name="pS")
                            for j in range(2):
                                sk = 2 * g + j
                                nc.tensor.matmul(
                                    pS[:, j, :],
                                    QKT[:, HL + h, sk * 128:(sk + 1) * 128],
                                    QKT[:, h, q0:q0 + 512],
                                    start=True, stop=True,
                                )
                            eS = ep.tile([128, 2, 512], bf16, tag="eS")
                            nc.scalar.activation(eS[:], pS[:], EXP,
                                                 bias=zbias[:, 0:1])
                            fifo.append((ci, g, eS))
                            if len(fifo) > LAG:
                                cci, cg, ceS = fifo.pop(0)
                                consume(k, cci, cg, ceS)
                            while pend_tail and pend_tail[0][0] <= k:
                                emit_tail_end(pend_tail.pop(0)[1])
                        k = len(stream)
                        while fifo:
                            cci, cg, ceS = fifo.pop(0)
                            consume(k, cci, cg, ceS)
                            k += 1
                        while pend_tail:
                            emit_tail_end(pend_tail.pop(0)[1])

                    # ---------------- phase 3: output projection ----------------
                    with nc.named_scope(f"ph3b{b}"), \
                         tc.tile_pool(name=f"p3y{b}", bufs=4) as yp3, \
                         tc.tile_pool(name=f"p3ps{b}", bufs=4, space="PSUM") as psY:
                        for nE in range(8):
                            wo_t = wo_sl[nE]
                            for m in range(16):
                                py = psY.tile([128, 512], fp32, tag="py", name="py")
                                for kd in range(HL):
                                    nc.tensor.matmul(
                                        py[:],
                                        OTT[:, m, kd, :],
                                        wo_t[:, kd, :],
                                        start=(kd == 0), stop=(kd == HL - 1),
                                    )
                                yt = yp3.tile([128, 512], bf16, tag="yt")
                                if m % 2 == 0:
                                    nc.scalar.copy(yt[:], py[:])
                                else:
                                    nc.vector.tensor_copy(yt[:], py[:])
                                nc.sync.dma_start(
                                    y_d[b * S + m * 128: b * S + (m + 1) * 128,
                                        nE * 512:(nE + 1) * 512],
                                    yt[:],
                                )
                            if nE + 2 < 8:
                                j = nE + 2
                                wo_sl[j] = wop.tile([128, HL, 512], bf16,
                                                    tag=f"wo{nE % 2}",
                                                    name=f"wo{b}_{j}")
                                nc.sync.dma_start(
                                    wo_sl[j][:],
                                    wo_d[:, :, j * 512:(j + 1) * 512])

    nc.compile()
    return nc


def _prep_inputs(x, freqs_cos, freqs_sin, wq, wk, wv, wo):
    x = np.asarray(x, np.float32)
    c = np.asarray(freqs_cos, np.float32)
    s = np.asarray(freqs_sin, np.float32)
    wq = np.asarray(wq, np.float32)
    wk = np.asarray(wk, np.float32)
    wv = np.asarray(wv, np.float32)
    wo = np.asarray(wo, np.float32)
    bf = ml_dtypes.bfloat16

    xT = x.reshape(T, E).T.astype(bf)
    xh = np.ascontiguousarray(
        xT.reshape(NKB, 128, T // CH1, CH1).transpose(2, 1, 0, 3))

    def fold(w):
        wr = w.reshape(H, D // 2, 2, E)
        w0, w1 = wr[:, :, 0], wr[:, :, 1]
        r0 = c[:, :, None] * w0 - s[:, :, None] * w1
        r1 = s[:, :, None] * w0 + c[:, :, None] * w1
        return np.stack([r0, r1], axis=2).reshape(E, E)

    wq_r = fold(wq) * np.float32(D ** -0.5)
    wk_r = fold(wk)

    in_maps = []
    for cix in range(NCORES):
        sl = slice(cix * W, (cix + 1) * W)
        qk = np.concatenate([wq_r[sl].T, wk_r[sl].T], axis=1)   # [E, 2W]
        wqkh = np.ascontiguousarray(
            qk.astype(bf).reshape(NKB, 128, 2 * W).transpose(1, 0, 2))
        wvh = np.ascontiguousarray(
            wv[sl].T.astype(bf).reshape(NKB, 128, W).transpose(1, 0, 2))
        woh = np.ascontiguousarray(
            wo[:, sl].T.astype(bf).reshape(HL, 128, E).transpose(1, 0, 2))
        in_maps.append({"xh": xh, "wqk": wqkh, "wv": wvh, "wo": woh})
    return in_maps


def run(x, freqs_cos, freqs_sin, wq, wk, wv, wo, trace=False, tmpdir=None):
    from concourse.bass_utils import run_bass_kernel_spmd

    if "nc" not in _CACHE:
        _CACHE["nc"] = _build_nc()
    nc = _CACHE["nc"]
    in_maps = _prep_inputs(x, freqs_cos, freqs_sin, wq, wk, wv, wo)
    res = run_bass_kernel_spmd(
        nc, in_maps, list(range(NCORES)), trace=trace, tmpdir=tmpdir
    )
    y = np.asarray(res.results[0]["y"], np.float32)
    for r in res.results[1:]:
        y = y + np.asarray(r["y"], np.float32)
    return y.reshape(B, S, E), res


def kernel(x, start_pos=0, freqs_cos=None, freqs_sin=None,
           wq=None, wk=None, wv=None, wo=None):
    y, _ = run(x, freqs_cos, freqs_sin, wq, wk, wv, wo)
    return y
